# revision 1
# baseline (speedup 1.0000x reference)
"""Sparse-attention kernel for nn_Attention_53558242181469, SPMD across 8 trn2 NeuronCores.

Sharding (per spec hint): the 48 total heads (4 branches x 12 sub-heads) are
split 6-per-core. Each core also gets the matching row-slices of Wq/Wk/WO, so
the three big matmuls are sharded too. Per-head work (scores, softmax+sink,
top-k retrieval, V_net MLP) is fully local; a single all-reduce (psum) after
the branch-partial output projection produces the branch sum, which every core
scales by 1/N_BR into the final mean.

Top-k(12) is computed without sort/gather primitives: 12 rounds of
(row-max, select, knock-out) build the top-k-masked probability matrix, and the
weighted key-sum becomes a plain matmul with the vanilla keys.
"""

import functools

import jax
import jax.numpy as jnp
import numpy as np

D_MODEL, N_HEAD, N_BR = 768, 12, 4
DH = D_MODEL // N_HEAD            # 64
H_TOT = N_BR * N_HEAD             # 48
K_RETR = 12
MLP_SCALE = np.pi / np.sqrt(3.0)
N_CORES = 8
HPC = H_TOT // N_CORES            # 6 heads per core
B, T = 2, 1024

_EPS = np.float32(np.finfo(np.float32).eps)


def _rmsnorm(x):
    return x * jax.lax.rsqrt(jnp.mean(x * x, axis=-1, keepdims=True) + _EPS)


@functools.partial(jax.pmap, axis_name="x",
                   static_broadcasted_argnums=())
def _core_fn(A, X, Wq_w, Wq_b, Wk_w, Wk_b, skew, wedge_bias, sink, v_null,
             fc_w, fc_b, proj_w, proj_b, WO_rows, wob_mean, cos, sin, causal):
    # Q projection for this core's 6 heads: (B,T,384) -> (B,6,T,64)
    q = (A @ Wq_w.T + Wq_b).reshape(B, T, HPC, DH).transpose(0, 2, 1, 3)
    q = _rmsnorm(q)
    # K projection for this core's 6 sub-heads (pre-wedge "vanilla" keys)
    k = (X @ Wk_w.T + Wk_b).reshape(B, T, HPC, DH).transpose(0, 2, 1, 3)
    k_vanilla = k

    # BiasedWedge: x + x @ skew + x * diag_bias  (skew shared, bias per head)
    qw = q + jnp.einsum("bhtd,de->bhte", q, skew) + q * wedge_bias[None, :, None, :]
    kw = k + jnp.einsum("bhtd,de->bhte", k, skew) + k * wedge_bias[None, :, None, :]

    # RoPE (interleaved halves concatenated)
    def rope(x):
        x1, x2 = x[..., 0::2], x[..., 1::2]
        return jnp.concatenate([x1 * cos - x2 * sin, x1 * sin + x2 * cos], axis=-1)

    qr, kr = rope(qw), rope(kw)

    scale = DH ** -0.5
    scores = jnp.einsum("bhtd,bhsd->bhts", qr, kr) * scale      # (B,6,T,T)
    scores = jnp.where(causal, -1e30, scores)

    # softmax over [scores, sink]
    sinks = jnp.broadcast_to(sink.reshape(1, HPC, 1, 1), (B, HPC, T, 1))
    m = jnp.maximum(jnp.max(scores, axis=-1, keepdims=True), sinks)
    e_tok = jnp.exp(scores - m)
    e_sink = jnp.exp(sinks - m)
    denom = jnp.sum(e_tok, axis=-1, keepdims=True) + e_sink
    probs_tok = jnp.where(causal, 0.0, e_tok / denom)
    probs_sink = e_sink / denom                                  # (B,6,T,1)

    # top-12 masked probabilities via iterative knock-out (no sort/gather)
    work = probs_tok
    masked = jnp.zeros_like(work)
    for _ in range(K_RETR):
        mx = jnp.max(work, axis=-1, keepdims=True)
        hit = (work == mx) & (mx > 0)
        masked = jnp.where(hit, work, masked)
        work = jnp.where(hit, -1.0, work)

    marker = (jnp.einsum("bhts,bhsd->bhtd", masked, k_vanilla) + k_vanilla) / (K_RETR + 1)

    # per-token V_net MLP on head_dim
    h = marker @ fc_w.T + fc_b
    h = h * h + 0.75 * h * h * h
    h = _rmsnorm(h)
    h = h * jax.nn.sigmoid(MLP_SCALE * h)
    out_tokens = h @ proj_w.T + proj_b                           # (B,6,T,64)

    context = out_tokens + probs_sink * v_null[None, :, None, :]
    # this core's 6 heads are contiguous sub-heads of ONE branch ->
    # a contiguous 384-column slice of that branch's (B,T,768) context
    ctx = context.transpose(0, 2, 1, 3).reshape(B, T, HPC * DH)

    y_part = ctx @ WO_rows                                       # (B,T,768)
    y = jax.lax.psum(y_part, "x")
    return y / N_BR + wob_mean


_CACHE = {}


def _shard_inputs(A, X, Wq_w, Wq_b, Wk_w, Wk_b, wedge_A, wedge_bias,
                  sink_scalars, v_nulls, fc_w, fc_b, proj_w, proj_b, WO, WO_b):
    skew = (wedge_A - wedge_A.T).astype(np.float32)
    inv_freq = 1.0 / (10000.0 ** (np.arange(0, DH, 2, dtype=np.float32) / DH))
    freqs = np.arange(T, dtype=np.float32)[:, None] * inv_freq[None, :]
    cos, sin = np.cos(freqs).astype(np.float32), np.sin(freqs).astype(np.float32)
    causal = np.triu(np.ones((T, T), bool), 1)
    wob_mean = WO_b.mean(axis=0).astype(np.float32)
    vn = v_nulls.reshape(H_TOT, DH)

    sh = {k: [] for k in ("Wq_w", "Wq_b", "Wk_w", "Wk_b", "wb", "sink", "vn", "WO")}
    for d in range(N_CORES):
        h0 = d * HPC                      # first head on this core
        br = h0 // N_HEAD                 # its branch
        s0 = h0 % N_HEAD                  # first sub-head within branch
        sh["Wq_w"].append(Wq_w[h0 * DH:(h0 + HPC) * DH])
        sh["Wq_b"].append(Wq_b[h0 * DH:(h0 + HPC) * DH])
        sh["Wk_w"].append(Wk_w[s0 * DH:(s0 + HPC) * DH])
        sh["Wk_b"].append(Wk_b[s0 * DH:(s0 + HPC) * DH])
        sh["wb"].append(wedge_bias[h0:h0 + HPC])
        sh["sink"].append(sink_scalars[h0:h0 + HPC])
        sh["vn"].append(vn[h0:h0 + HPC])
        sh["WO"].append(WO[br, s0 * DH:(s0 + HPC) * DH, :])

    def rep(x):
        x = np.asarray(x, np.float32)
        return np.broadcast_to(x, (N_CORES,) + x.shape)

    def stk(key):
        return np.ascontiguousarray(np.stack(sh[key]).astype(np.float32))

    return (rep(A), rep(X), stk("Wq_w"), stk("Wq_b"), stk("Wk_w"), stk("Wk_b"),
            rep(skew), stk("wb"), stk("sink"), stk("vn"),
            rep(fc_w), rep(fc_b), rep(proj_w), rep(proj_b), stk("WO"),
            rep(wob_mean), rep(cos), rep(sin),
            np.broadcast_to(causal, (N_CORES, T, T)))


def kernel(**inputs) -> np.ndarray:
    args = _shard_inputs(**{k: np.asarray(v) for k, v in inputs.items()})
    y = _core_fn(*args)
    return np.asarray(y[0], dtype=np.float32)



# revision 8
# speedup vs baseline: 2.9626x; 2.9626x over previous
"""Sparse-attention Bass/Tile kernel for nn_Attention_53558242181469.

SPMD over 8 NeuronCores: the 48 heads (4 branches x 12 sub-heads) are split
6-per-core (each core owns 6 contiguous sub-heads of one branch), so Wq/Wk/WO
are row-sharded too.  Per-core, everything runs as one Bass/Tile program:

  * Q/K projections in transposed layout (d on partitions) off PE-transposed
    A^T/X^T; q's rmsnorm is folded into the per-row exp scale
    (1/sqrt(ssq+64*eps) absorbs both rsqrt(mean+eps) and dh^-0.5).
  * BiasedWedge as a single 64x64 matmul (I + S^T, host-permuted), RoPE as
    elementwise mul/add against host-built [32,1024] cos/sin tables (head dims
    pre-permuted to even/odd halves via the weight rows).
  * scores = qr^T @ kr per 128-row block (causal: only the lower-triangular
    column blocks are computed); exp on the Scalar engine with accum_out
    giving the softmax denominator for free; sink handled as exp(sink) const.
  * top-12 via the DVE max8/match_replace instructions: knock out top-8, then
    ranks 9-12, and masked = e - knocked (exact knockout semantics, tie-safe).
  * the (top12-masked, normalized) prob matrix is PE-transposed per 128x128
    block and contracted with the vanilla keys; +kv/13 enters via an I/13 add
    on the diagonal block.
  * V_net MLP in transposed layout; rmsnorm-over-256 via ones-matmul +
    rank-1 broadcast; h*sigmoid(c*h) as Silu(c*h) with proj_w pre-scaled.
  * per-branch W_O applied per row block (ctx stacked 6 heads -> 384 rows),
    WO_b.mean/8 added via a rank-1 matmul; ReduceScatter over the 8 cores
    sums branches; each core emits its 256-row slice of the (2048,768) output.
"""

import numpy as np

import concourse.bass as bass
import concourse.mybir as mybir
import concourse.tile as tile
from concourse import bacc
from concourse.bass_utils import run_bass_kernel_spmd
from concourse.masks import make_causal_mask, make_identity

F32 = mybir.dt.float32
AF = mybir.ActivationFunctionType
ALU = mybir.AluOpType

D_MODEL, N_HEAD, N_BR = 768, 12, 4
DH = 64
H_TOT = 48
K_RETR = 12
MLP_SCALE = float(np.pi / np.sqrt(3.0))
N_CORES, HPC = 8, 6
B, T = 2, 1024
NRB = T // 128                      # 8 row blocks per (b)
EPS = float(np.finfo(np.float32).eps)
PERM = np.concatenate([np.arange(0, DH, 2), np.arange(1, DH, 2)])
NEG = -1.0e30


def _emit(tc, io):
    nc = tc.nc
    ctx_mgr_pools = []

    cpool = tc.alloc_tile_pool(name="const", bufs=1)
    dpool = tc.alloc_tile_pool(name="dram", bufs=1, space="DRAM")
    psum = tc.alloc_tile_pool(name="psum", bufs=2, space="PSUM")
    sb1 = tc.alloc_tile_pool(name="sb1", bufs=1)
    sb2 = tc.alloc_tile_pool(name="sb2", bufs=2)
    sb3 = tc.alloc_tile_pool(name="sb3", bufs=3)

    # ---------------- constants -> SBUF ----------------
    wqt = [cpool.tile([128, HPC * DH], F32, name=f"wqt{ci}") for ci in range(6)]
    wkt = [cpool.tile([128, HPC * DH], F32, name=f"wkt{ci}") for ci in range(6)]
    for ci in range(6):
        nc.sync.dma_start(wqt[ci], io["WqT"][ci * 128:(ci + 1) * 128, :])
        nc.sync.dma_start(wkt[ci], io["WkT"][ci * 128:(ci + 1) * 128, :])
    qb_t = cpool.tile([DH, HPC], F32, name="qb_t")
    kb_t = cpool.tile([DH, HPC], F32, name="kb_t")
    nc.sync.dma_start(qb_t, io["QB"][:, :])
    nc.sync.dma_start(kb_t, io["KB"][:, :])
    wedge_t = cpool.tile([DH, HPC * DH], F32, name="wedge_t")
    nc.sync.dma_start(wedge_t, io["WEDGE"][:, :])
    cos_t = cpool.tile([32, T], F32, name="cos_t")
    sin_t = cpool.tile([32, T], F32, name="sin_t")
    nc.sync.dma_start(cos_t, io["COS"][:, :])
    nc.sync.dma_start(sin_t, io["SIN"][:, :])
    esink_t = cpool.tile([128, HPC], F32, name="esink_t")
    nc.sync.dma_start(esink_t, io["ESINK"][:, :])
    vns_t = cpool.tile([1, HPC * DH], F32, name="vns_t")
    nc.sync.dma_start(vns_t, io["VNS"][:, :])
    fa_t = cpool.tile([DH + 1, 256], F32, name="fa_t")
    nc.sync.dma_start(fa_t, io["FA"][:, :])
    ptp_t = cpool.tile([128, 128], F32, name="ptp_t")
    nc.sync.dma_start(ptp_t, io["PTP"][:, :])
    pb_t = cpool.tile([DH, 1], F32, name="pb_t")
    nc.sync.dma_start(pb_t, io["PB"][:, :])
    wo_t = [cpool.tile([128, D_MODEL], F32, name=f"wo{ci}") for ci in range(3)]
    for ci in range(3):
        nc.sync.dma_start(wo_t[ci], io["WOr"][ci * 128:(ci + 1) * 128, :])
    wob8_t = cpool.tile([1, D_MODEL], F32, name="wob8_t")
    nc.sync.dma_start(wob8_t, io["WOB8"][:, :])

    ident = cpool.tile([128, 128], F32, name="ident")
    make_identity(nc, ident)
    ident13 = cpool.tile([128, 128], F32, name="ident13")
    nc.gpsimd.memset(ident13, 0.0)
    nc.gpsimd.affine_select(
        out=ident13, in_=ident13, compare_op=ALU.not_equal,
        fill=1.0 / (K_RETR + 1.0), base=0, pattern=[[-1, 128]],
        channel_multiplier=1)
    cmask = cpool.tile([128, 128], F32, name="cmask")
    make_causal_mask(nc, cmask, mask_val=NEG)
    ones_row = cpool.tile([1, 128], F32, name="ones_row")
    nc.gpsimd.memset(ones_row, 1.0)
    ones_col = cpool.tile([128, 1], F32, name="ones_col")
    nc.gpsimd.memset(ones_col, 1.0)
    c64eps = cpool.tile([128, 1], F32, name="c64eps")
    nc.gpsimd.memset(c64eps, float(DH) * EPS)
    ceps = cpool.tile([1, 1], F32, name="ceps")
    nc.gpsimd.memset(ceps, EPS)

    ybounce = dpool.tile([B * T, D_MODEL], F32, name="ybounce")
    yrs = dpool.tile([B * T // N_CORES, D_MODEL], F32, name="yrs")

    # ---------------- main program ----------------
    for b in range(B):
        # A^T / X^T via PE transposes of row tiles
        at = [sb1.tile([128, T], F32, name=f"at{ci}", tag=f"at{ci}") for ci in range(6)]
        xt = [sb1.tile([128, T], F32, name=f"xt{ci}", tag=f"xt{ci}") for ci in range(6)]
        for src, dst in ((io["A"], at), (io["X"], xt)):
            for rt in range(NRB):
                arow = sb3.tile([128, D_MODEL], F32, name="arow", tag="arow")
                nc.sync.dma_start(arow, src[b, rt * 128:(rt + 1) * 128, :])
                for ci in range(6):
                    tp = psum.tile([128, 128], F32, name="tpa", tag="sm")
                    nc.tensor.transpose(tp, arow[:, ci * 128:(ci + 1) * 128], ident)
                    eng = nc.scalar if ci % 2 == 0 else nc.vector
                    if ci % 2 == 0:
                        nc.scalar.copy(dst[ci][:, rt * 128:(rt + 1) * 128], tp)
                    else:
                        nc.vector.tensor_copy(dst[ci][:, rt * 128:(rt + 1) * 128], tp)

        ctx_tiles = [sb1.tile([128, 3 * 128], F32, name=f"ctx{rb}", tag=f"ctx{rb}")
                     for rb in range(NRB)]

        for h in range(HPC):
            hs = slice(h * DH, (h + 1) * DH)
            # ---- Q projection (transposed, permuted) ----
            qp = psum.tile([DH, T], F32, name="qp", tag="mm")
            for nh in range(2):
                ns = slice(nh * 512, (nh + 1) * 512)
                for ci in range(6):
                    nc.tensor.matmul(qp[:, ns], wqt[ci][:, hs], at[ci][:, ns],
                                     start=(ci == 0), stop=(ci == 5))
            q_sb = sb1.tile([DH, T], F32, name="q_sb", tag="q_sb")
            nc.scalar.activation(q_sb, qp, AF.Identity, bias=qb_t[:, h:h + 1])
            sq = sb1.tile([DH, T], F32, name="sq", tag="sq")
            nc.scalar.activation(sq, qp, AF.Square, bias=qb_t[:, h:h + 1])
            ssq_ps = psum.tile([128, NRB], F32, name="ssq_ps", tag="sm")
            for rb in range(NRB):
                nc.tensor.matmul(ssq_ps[:, rb:rb + 1],
                                 sq[:, rb * 128:(rb + 1) * 128],
                                 ones_col[0:DH, :], start=True, stop=True)
            r8 = sb2.tile([128, NRB], F32, name="r8", tag="r8")
            nc.scalar.activation(r8, ssq_ps, AF.Sqrt, bias=c64eps)
            s8 = sb2.tile([128, NRB], F32, name="s8", tag="s8")
            nc.vector.reciprocal(s8, r8)

            # ---- wedge + rope for q ----
            qr = sb2.tile([DH, T], F32, name="qr", tag="qr")
            wp = psum.tile([DH, T], F32, name="wp", tag="mm")
            for nh in range(2):
                ns = slice(nh * 512, (nh + 1) * 512)
                nc.tensor.matmul(wp[:, ns], wedge_t[:, hs], q_sb[:, ns],
                                 start=True, stop=True)
            pa = sb2.tile([32, T], F32, name="pa", tag="ropetmp")
            pb2 = sb2.tile([32, T], F32, name="pb2", tag="ropetmp")
            nc.vector.tensor_mul(pa, wp[0:32, :], cos_t)
            nc.vector.tensor_mul(pb2, wp[32:64, :], sin_t)
            nc.gpsimd.tensor_sub(qr[0:32, :], pa, pb2)
            pc = sb2.tile([32, T], F32, name="pc", tag="ropetmp")
            pd = sb2.tile([32, T], F32, name="pd", tag="ropetmp")
            nc.vector.tensor_mul(pc, wp[0:32, :], sin_t)
            nc.vector.tensor_mul(pd, wp[32:64, :], cos_t)
            nc.gpsimd.tensor_add(qr[32:64, :], pc, pd)

            # ---- K projection (vanilla) + wedge + rope ----
            kp = psum.tile([DH, T], F32, name="kp", tag="mm")
            for nh in range(2):
                ns = slice(nh * 512, (nh + 1) * 512)
                for ci in range(6):
                    nc.tensor.matmul(kp[:, ns], wkt[ci][:, hs], xt[ci][:, ns],
                                     start=(ci == 0), stop=(ci == 5))
            kv_sb = sb2.tile([DH, T], F32, name="kv_sb", tag="kv_sb")
            nc.scalar.activation(kv_sb, kp, AF.Identity, bias=kb_t[:, h:h + 1])
            kr = sb2.tile([DH, T], F32, name="kr", tag="kr")
            wpk = psum.tile([DH, T], F32, name="wpk", tag="mm")
            for nh in range(2):
                ns = slice(nh * 512, (nh + 1) * 512)
                nc.tensor.matmul(wpk[:, ns], wedge_t[:, hs], kv_sb[:, ns],
                                 start=True, stop=True)
            ka = sb2.tile([32, T], F32, name="ka", tag="ropetmp")
            kb2 = sb2.tile([32, T], F32, name="kb2", tag="ropetmp")
            nc.vector.tensor_mul(ka, wpk[0:32, :], cos_t)
            nc.vector.tensor_mul(kb2, wpk[32:64, :], sin_t)
            nc.gpsimd.tensor_sub(kr[0:32, :], ka, kb2)
            kc = sb2.tile([32, T], F32, name="kc", tag="ropetmp")
            kd = sb2.tile([32, T], F32, name="kd", tag="ropetmp")
            nc.vector.tensor_mul(kc, wpk[0:32, :], sin_t)
            nc.vector.tensor_mul(kd, wpk[32:64, :], cos_t)
            nc.gpsimd.tensor_add(kr[32:64, :], kc, kd)

            # ---- kv in row layout (for the marker contraction) ----
            kvrow = sb1.tile([128, NRB * DH], F32, name="kvrow", tag="kvrow")
            for j in range(NRB):
                tpk = psum.tile([128, DH], F32, name="tpk", tag="sm")
                nc.tensor.transpose(tpk, kv_sb[:, j * 128:(j + 1) * 128],
                                    ident[0:DH, 0:DH])
                nc.scalar.copy(kvrow[:, j * DH:(j + 1) * DH], tpk)

            marker_sb = sb1.tile([DH + 1, T], F32, name="marker_sb", tag="marker")
            nc.gpsimd.memset(marker_sb[DH:DH + 1, :], 1.0)

            for rb in range(NRB):
                W = 128 * (rb + 1)
                ds = slice(rb * 128, W)
                # ---- scores ----
                sc_ps = psum.tile([128, T], F32, name="sc_ps", tag="mm")
                for n0 in range(0, W, 512):
                    nw = min(512, W - n0)
                    nc.tensor.matmul(sc_ps[:, n0:n0 + nw], qr[:, ds],
                                     kr[:, n0:n0 + nw], start=True, stop=True)
                nc.vector.tensor_add(sc_ps[:, ds], sc_ps[:, ds], cmask)
                # ---- exp + denom ----
                e_t = sb2.tile([128, T], F32, name="e_t", tag="e_t")
                acc = sb2.tile([128, 1], F32, name="acc", tag="acc")
                nc.scalar.activation(e_t[:, 0:W], sc_ps[:, 0:W], AF.Exp,
                                     scale=s8[:, rb:rb + 1], accum_out=acc)
                denom = sb2.tile([128, 1], F32, name="denom", tag="denom")
                nc.vector.tensor_scalar(denom, acc, esink_t[:, h:h + 1], None,
                                        op0=ALU.add)
                recip = sb2.tile([128, 1], F32, name="recip", tag="recip")
                nc.vector.reciprocal(recip, denom)
                # ---- top-12 knockout ----
                m8a = sb2.tile([128, 8], F32, name="m8a", tag="m8a")
                nc.vector.max(out=m8a, in_=e_t[:, 0:W])
                work = sb2.tile([128, T], F32, name="work", tag="work")
                nc.vector.match_replace(out=work[:, 0:W], in_to_replace=m8a,
                                        in_values=e_t[:, 0:W], imm_value=0.0)
                m8b = sb2.tile([128, 8], F32, name="m8b", tag="m8b")
                nc.vector.max(out=m8b, in_=work[:, 0:W])
                m2b = sb2.tile([128, 8], F32, name="m2b", tag="m2b")
                nc.gpsimd.memset(m2b[:, 4:8], -1.0)
                nc.vector.tensor_copy(m2b[:, 0:4], m8b[:, 0:4])
                nc.vector.match_replace(out=work[:, 0:W], in_to_replace=m2b,
                                        in_values=work[:, 0:W], imm_value=0.0)
                masked = sb2.tile([128, T], F32, name="masked", tag="masked")
                nc.vector.tensor_sub(masked[:, 0:W], e_t[:, 0:W], work[:, 0:W])
                nc.vector.tensor_scalar(masked[:, 0:W], masked[:, 0:W], recip,
                                        1.0 / (K_RETR + 1.0),
                                        op0=ALU.mult, op1=ALU.mult)
                # ---- probs_sink row ----
                srow_ps = psum.tile([1, 128], F32, name="srow_ps", tag="sm")
                nc.tensor.matmul(srow_ps, recip, ident, start=True, stop=True)
                reciprow = sb2.tile([1, 128], F32, name="reciprow", tag="reciprow")
                nc.scalar.copy(reciprow, srow_ps)
                # ---- marker = (masked @ kv + kv)/13, transposed ----
                mk_ps = psum.tile([DH, 128], F32, name="mk_ps", tag="mk")
                for j in range(rb + 1):
                    tpm = psum.tile([128, 128], F32, name="tpm", tag="sm")
                    nc.tensor.transpose(tpm, masked[:, j * 128:(j + 1) * 128], ident)
                    mT = sb3.tile([128, 128], F32, name="mT", tag="mT")
                    if j == rb:
                        nc.vector.tensor_add(mT, tpm, ident13)
                    else:
                        nc.scalar.copy(mT, tpm)
                    nc.tensor.matmul(mk_ps, kvrow[:, j * DH:(j + 1) * DH], mT,
                                     start=(j == 0), stop=(j == rb))
                nc.scalar.copy(marker_sb[0:DH, ds], mk_ps)
                # ---- V_net MLP (transposed) ----
                h1_ps = psum.tile([128, 256], F32, name="h1_ps", tag="sm")
                nc.tensor.matmul(h1_ps[:, 0:128], fa_t[:, 0:128],
                                 marker_sb[:, ds], start=True, stop=True)
                nc.tensor.matmul(h1_ps[:, 128:256], fa_t[:, 128:256],
                                 marker_sb[:, ds], start=True, stop=True)
                s1 = sb2.tile([128, 256], F32, name="s1", tag="s1")
                nc.scalar.activation(s1, h1_ps, AF.Copy, bias=1.0, scale=0.75)
                sqm = sb2.tile([128, 256], F32, name="sqm", tag="sqm")
                nc.scalar.activation(sqm, h1_ps, AF.Square)
                hp = sb2.tile([128, 256], F32, name="hp", tag="hp")
                nc.vector.tensor_mul(hp, sqm, s1)
                sq2 = sb2.tile([128, 256], F32, name="sq2", tag="sq2")
                nc.scalar.activation(sq2, hp, AF.Square)
                mss_ps = psum.tile([1, 128], F32, name="mss_ps", tag="sm")
                nc.tensor.matmul(mss_ps, ones_col, sq2[:, 0:128],
                                 start=True, stop=False)
                nc.tensor.matmul(mss_ps, ones_col, sq2[:, 128:256],
                                 start=False, stop=True)
                rmsrow = sb2.tile([1, 128], F32, name="rmsrow", tag="rmsrow")
                nc.scalar.activation(rmsrow, mss_ps, AF.Sqrt, bias=ceps,
                                     scale=1.0 / 256.0)
                rmscol_ps = psum.tile([128, 1], F32, name="rmscol_ps", tag="sm")
                nc.tensor.matmul(rmscol_ps, rmsrow, ones_row[0:1, 0:1],
                                 start=True, stop=True)
                invcol = sb2.tile([128, 1], F32, name="invcol", tag="invcol")
                nc.vector.reciprocal(invcol, rmscol_ps)
                invrow_ps = psum.tile([1, 128], F32, name="invrow_ps", tag="sm")
                nc.tensor.matmul(invrow_ps, invcol, ident, start=True, stop=True)
                invrow = sb2.tile([1, 128], F32, name="invrow", tag="invrow")
                nc.scalar.copy(invrow, invrow_ps)
                invbc_ps = psum.tile([128, 128], F32, name="invbc_ps", tag="sm")
                nc.tensor.matmul(invbc_ps, ones_row, invrow, start=True, stop=True)
                hn = sb2.tile([128, 256], F32, name="hn", tag="hn")
                nc.vector.tensor_mul(hn[:, 0:128], hp[:, 0:128], invbc_ps)
                nc.vector.tensor_mul(hn[:, 128:256], hp[:, 128:256], invbc_ps)
                hf = sb2.tile([128, 256], F32, name="hf", tag="hf")
                nc.scalar.activation(hf, hn, AF.Silu, scale=MLP_SCALE)
                ot_ps = psum.tile([DH, 128], F32, name="ot_ps", tag="mk")
                nc.tensor.matmul(ot_ps, ptp_t[:, 0:DH], hf[:, 0:128],
                                 start=True, stop=False)
                nc.tensor.matmul(ot_ps, ptp_t[:, DH:128], hf[:, 128:256],
                                 start=False, stop=False)
                nc.tensor.matmul(ot_ps, vns_t[0:1, hs], reciprow,
                                 start=False, stop=True)
                nc.scalar.activation(
                    ctx_tiles[rb][DH * (h % 2):DH * (h % 2) + DH,
                                  128 * (h // 2):128 * (h // 2) + 128],
                    ot_ps, AF.Identity, bias=pb_t)

        # ---- output projection + bias, per row block ----
        for rb in range(NRB):
            y_ps = psum.tile([128, D_MODEL], F32, name="y_ps", tag="mm")
            for n0, nw in ((0, 512), (512, 256)):
                for ci in range(3):
                    nc.tensor.matmul(y_ps[:, n0:n0 + nw],
                                     ctx_tiles[rb][:, ci * 128:(ci + 1) * 128],
                                     wo_t[ci][:, n0:n0 + nw],
                                     start=(ci == 0), stop=False)
                nc.tensor.matmul(y_ps[:, n0:n0 + nw], ones_row,
                                 wob8_t[0:1, n0:n0 + nw], start=False, stop=True)
            y_sb = sb2.tile([128, D_MODEL], F32, name="y_sb", tag="y_sb")
            nc.vector.tensor_copy(y_sb, y_ps)
            nc.sync.dma_start(
                ybounce[b * T + rb * 128: b * T + (rb + 1) * 128, :], y_sb)

    nc.gpsimd.collective_compute(
        "ReduceScatter", ALU.add, replica_groups=[list(range(N_CORES))],
        ins=[ybounce.opt()], outs=[yrs.opt()])
    nc.sync.dma_start(io["Y"][:, :], yrs)

    for p in (sb3, sb2, sb1, psum, dpool, cpool):
        p.release()


_CACHE = {}


def _build():
    if "nc" in _CACHE:
        return _CACHE["nc"]
    nc = bacc.Bacc("TRN2", target_bir_lowering=False, debug=False,
                   num_devices=N_CORES, enable_asserts=False)
    io = {
        "A": nc.dram_tensor("A", [B, T, D_MODEL], F32, kind="ExternalInput"),
        "X": nc.dram_tensor("X", [B, T, D_MODEL], F32, kind="ExternalInput"),
        "WqT": nc.dram_tensor("WqT", [D_MODEL, HPC * DH], F32, kind="ExternalInput"),
        "QB": nc.dram_tensor("QB", [DH, HPC], F32, kind="ExternalInput"),
        "WkT": nc.dram_tensor("WkT", [D_MODEL, HPC * DH], F32, kind="ExternalInput"),
        "KB": nc.dram_tensor("KB", [DH, HPC], F32, kind="ExternalInput"),
        "WEDGE": nc.dram_tensor("WEDGE", [DH, HPC * DH], F32, kind="ExternalInput"),
        "COS": nc.dram_tensor("COS", [32, T], F32, kind="ExternalInput"),
        "SIN": nc.dram_tensor("SIN", [32, T], F32, kind="ExternalInput"),
        "ESINK": nc.dram_tensor("ESINK", [128, HPC], F32, kind="ExternalInput"),
        "VNS": nc.dram_tensor("VNS", [1, HPC * DH], F32, kind="ExternalInput"),
        "FA": nc.dram_tensor("FA", [DH + 1, 256], F32, kind="ExternalInput"),
        "PTP": nc.dram_tensor("PTP", [128, 128], F32, kind="ExternalInput"),
        "PB": nc.dram_tensor("PB", [DH, 1], F32, kind="ExternalInput"),
        "WOr": nc.dram_tensor("WOr", [HPC * DH, D_MODEL], F32, kind="ExternalInput"),
        "WOB8": nc.dram_tensor("WOB8", [1, D_MODEL], F32, kind="ExternalInput"),
        "Y": nc.dram_tensor("Y", [B * T // N_CORES, D_MODEL], F32,
                            kind="ExternalOutput"),
    }
    with tile.TileContext(nc) as tc:
        _emit(tc, io)
    nc.compile()
    _CACHE["nc"] = nc
    return nc


def _prep_in_maps(inputs):
    A = np.ascontiguousarray(np.asarray(inputs["A"], np.float32))
    X = np.ascontiguousarray(np.asarray(inputs["X"], np.float32))
    Wq_w = np.asarray(inputs["Wq_w"], np.float32)
    Wq_b = np.asarray(inputs["Wq_b"], np.float32)
    Wk_w = np.asarray(inputs["Wk_w"], np.float32)
    Wk_b = np.asarray(inputs["Wk_b"], np.float32)
    wedge_A = np.asarray(inputs["wedge_A"], np.float32)
    wb = np.asarray(inputs["wedge_bias"], np.float32)
    sink = np.asarray(inputs["sink_scalars"], np.float32).reshape(H_TOT)
    v_nulls = np.asarray(inputs["v_nulls"], np.float32).reshape(H_TOT, DH)
    fc_w = np.asarray(inputs["fc_w"], np.float32)
    fc_b = np.asarray(inputs["fc_b"], np.float32)
    proj_w = np.asarray(inputs["proj_w"], np.float32)
    proj_b = np.asarray(inputs["proj_b"], np.float32)
    WO = np.asarray(inputs["WO"], np.float32)
    WO_b = np.asarray(inputs["WO_b"], np.float32)

    skew = wedge_A - wedge_A.T
    inv_freq = 1.0 / (10000.0 ** (np.arange(0, DH, 2, dtype=np.float32) / DH))
    freqs = np.arange(T, dtype=np.float32)[:, None] * inv_freq[None, :]
    COS = np.ascontiguousarray(np.cos(freqs).T.astype(np.float32))
    SIN = np.ascontiguousarray(np.sin(freqs).T.astype(np.float32))
    FA = np.concatenate([fc_w[:, PERM].T, fc_b[None, :]], axis=0)
    FA = np.ascontiguousarray(FA.astype(np.float32))
    PT = (proj_w / MLP_SCALE).T.astype(np.float32)        # [256, 64]
    PTP = np.concatenate([PT[0:128], PT[128:256]], axis=1)
    PTP = np.ascontiguousarray(PTP)
    PB = np.ascontiguousarray(proj_b[:, None].astype(np.float32))
    WOB8 = np.ascontiguousarray((WO_b.mean(axis=0) / N_CORES)[None, :]
                                .astype(np.float32))
    eye = np.eye(DH, dtype=np.float32)

    in_maps = []
    for c in range(N_CORES):
        h0 = c * HPC
        br = h0 // N_HEAD
        s0 = h0 % N_HEAD
        rq = np.concatenate([(h0 + h) * DH + PERM for h in range(HPC)])
        rk = np.concatenate([(s0 + h) * DH + PERM for h in range(HPC)])
        WqT = np.ascontiguousarray(Wq_w[rq].T)            # [768, 384]
        QB = np.ascontiguousarray(Wq_b[rq].reshape(HPC, DH).T)  # [64, 6]
        WkT = np.ascontiguousarray(Wk_w[rk].T)
        KB = np.ascontiguousarray(Wk_b[rk].reshape(HPC, DH).T)
        wedges = []
        for h in range(HPC):
            g = h0 + h
            S_h = skew + np.diag(wb[g])
            Ww = (eye + S_h.T)[PERM][:, PERM]             # [64, 64]
            wedges.append(Ww.T)
        WEDGE = np.ascontiguousarray(np.concatenate(wedges, axis=1))
        es = np.exp(sink[h0:h0 + HPC]).astype(np.float32)
        ESINK = np.ascontiguousarray(np.broadcast_to(es[None, :], (128, HPC)))
        VNS = np.ascontiguousarray(
            (v_nulls[h0:h0 + HPC] * es[:, None]).reshape(1, HPC * DH))
        WOr = np.ascontiguousarray(
            WO[br, s0 * DH:(s0 + HPC) * DH, :] / float(N_BR))
        in_maps.append({
            "A": A, "X": X, "WqT": WqT, "QB": QB, "WkT": WkT, "KB": KB,
            "WEDGE": WEDGE, "COS": COS, "SIN": SIN, "ESINK": ESINK,
            "VNS": VNS, "FA": FA, "PTP": PTP, "PB": PB, "WOr": WOr,
            "WOB8": WOB8,
        })
    return in_maps


def run(inputs, **kwargs):
    nc = _build()
    in_maps = _prep_in_maps(inputs)
    res = run_bass_kernel_spmd(nc, in_maps, core_ids=list(range(N_CORES)),
                               **kwargs)
    parts = [res.results[c]["Y"] for c in range(N_CORES)]
    y = np.concatenate(parts, axis=0).reshape(B, T, D_MODEL)
    return y.astype(np.float32), res


def kernel(**inputs) -> np.ndarray:
    y, _ = run(inputs)
    return y
